# revision 33
# baseline (speedup 1.0000x reference)
"""Multi-head attention with RoPE on 8 Trainium2 NeuronCores — v2.

Sharding: core c handles batch b = c//2 and head-group hg = c%2 (8 of 16
heads).  Data-parallel over batch, tensor-parallel over heads; the
row-parallel wo all-reduce (2 cores per batch) happens on the host during
the gather/unshard step.

v2 redesign vs the baseline (711us):
  * bf16 for QT/KT/at/Vg/A/wo: every attention matmul runs at 1 cycle/row
    regardless of moving width (f32r drops to 4 cyc/row below N=256), and
    LDWEIGHTS gets the 4x fast-weight-load path.  Projections stay f32r.
  * Attention loops j(k-block)-outer, q-chunk inner, per q-half, so one
    LDWEIGHTS + one [128,<=1024] exp serves up to two q-chunks (24 ACT
    instructions per head instead of 40), and the PE stream has no
    long dependency stalls (HAM clock-gate stays warm).
  * The per-(h,qc) [1,512] DVE reciprocal (3.3us each, 107us total!) is
    replaced by a per-half [2,512] reciprocal_approx_fast + gpsimd
    partition_broadcast; the PE ones-outer-product broadcast and its PSUM
    bank are gone.
  * QT/KT projections for head-tiles 1-3 are emitted as filler inside the
    head-0/1 attention stream so the PE chews on them while ACT runs exp.
  * PSUM: one pool, tags "ps" ([128,1024] x2 = 4 banks, shared by
    projections / scores / wo) and "av" ([65,512] x4 = 4 banks).
"""

import sys
import types

sys.path.insert(0, "/opt/trn_rl_repo")

import numpy as np

import concourse.bacc as bacc
import concourse.mybir as mybir
import concourse.tile as tile
from concourse.bass_utils import run_bass_kernel_spmd

# Problem constants (hardcoded per contract)
B, S, D = 4, 2048, 1024
H = 16
DH = D // H          # 64
THETA = 10000.0
NCORES = 8
HG = 2               # head groups (tensor-parallel factor)
HD = D // HG         # 512 = per-core heads dim
NH = H // HG         # 8 heads per core
P = 128
SC = 512             # q-chunk (one PSUM bank of f32)
NSC = S // SC        # 4
NKB = S // P         # 16 k-blocks
NDB = D // P         # 8 d-blocks (contraction for projections)
SCALE = 1.0 / np.sqrt(np.float32(DH))

F32 = mybir.dt.float32
F32R = mybir.dt.float32r
BF16 = mybir.dt.bfloat16


def _install_ntff_hook():
    """Best-effort: register the axon NTFF profile hook so trace=True works."""
    try:
        import antenv

        if "antenv.axon_hooks" in sys.modules:
            return
        sys.path.insert(0, "/root/.axon_site/trn_agent_boot")
        import trn_boot

        hook = trn_boot._ntff_profile_via_ctypes("/opt/axon/libaxon_pjrt.so")
        mod = types.ModuleType("antenv.axon_hooks")
        mod.get_axon_ntff_profile_hook = lambda: hook
        mod.set_axon_ntff_profile_hook = lambda h: None
        sys.modules["antenv.axon_hooks"] = mod
        antenv.axon_hooks = mod
    except Exception:
        pass


def build_program(phase="full"):
    nc = bacc.Bacc("TRN2", target_bir_lowering=False, debug=False,
                   num_devices=NCORES)

    xt_d = nc.dram_tensor("xt", [D, S], F32R, kind="ExternalInput")
    wqt_d = nc.dram_tensor("wqt", [D, HD], F32R, kind="ExternalInput")
    wkt_d = nc.dram_tensor("wkt", [D, HD], F32R, kind="ExternalInput")
    wvt_d = nc.dram_tensor("wvt", [D, HD], F32R, kind="ExternalInput")
    wot_d = nc.dram_tensor("wot", [HD, D], F32, kind="ExternalInput")
    cf_d = nc.dram_tensor("cfull", [P, S], F32, kind="ExternalInput")
    sf_d = nc.dram_tensor("sfull", [P, S], F32, kind="ExternalInput")
    tri_d = nc.dram_tensor("tri", [P, P], F32, kind="ExternalInput")
    out_d = nc.dram_tensor("outT", [D, S], F32, kind="ExternalOutput")

    EXP = mybir.ActivationFunctionType.Exp
    COPY = mybir.ActivationFunctionType.Copy
    MULT = mybir.AluOpType.mult
    ADD = mybir.AluOpType.add

    with tile.TileContext(nc) as tc:
        with (
            nc.allow_low_precision(reason="bf16 attention, 2e-2 tolerance"),
            tc.tile_pool(name="big", bufs=8) as big,        # xt [128,2048] f32r
            tc.tile_pool(name="qk", bufs=1) as qk,          # QT/KT bf16 persistent
            tc.tile_pool(name="ap", bufs=1) as apool,       # A bf16 persistent
            tc.tile_pool(name="vg", bufs=1) as vgp,         # V_aug bf16 persistent
            tc.tile_pool(name="wv", bufs=8) as wvp,         # wv [128,512] f32r
            tc.tile_pool(name="wm", bufs=16) as wmp,        # wq/wk [128,128] slices
            tc.tile_pool(name="wo", bufs=1) as wop,         # wo bf16 persistent
            tc.tile_pool(name="cs", bufs=1) as csp,         # cos/sin bf16
            tc.tile_pool(name="sw", bufs=2) as swp,         # rope swap staging
            tc.tile_pool(name="at", bufs=3) as atp,         # exp(scores) bf16
            tc.tile_pool(name="ot", bufs=2) as otp,         # output staging f32
            tc.tile_pool(name="small", bufs=2) as small,    # dn/rcp/bc/tri
            tc.tile_pool(name="ps", bufs=1, space="PSUM") as psp,
        ):
            # ---- upfront DMAs ----
            # x^T in column-chunk order so the first projection unit can
            # start after ~4MB instead of the full 8MB.
            xt = [big.tile([P, S], F32R, tag="big", name=f"xt{k}")
                  for k in range(NDB)]
            for n in range(NSC):
                for k in range(NDB):
                    nc.sync.dma_start(xt[k][:, SC * n:SC * (n + 1)],
                                      xt_d[P * k:P * (k + 1), SC * n:SC * (n + 1)])
            # f32 inputs staged through `ot` tiles and cast to bf16 on-device
            def load_bf16(dst_ap, src_ap):
                st = otp.tile(list(src_ap.shape), F32, tag="ot",
                              padded_shape=[P, 2 * SC], name="st")
                nc.sync.dma_start(st[:], src_ap)
                nc.vector.tensor_copy(dst_ap, st[:])

            tri = small.tile([P, P], BF16, tag="tri", bufs=1)
            load_bf16(tri[:], tri_d[:])
            cf = csp.tile([P, S], BF16, tag="cf")
            sf = csp.tile([P, S], BF16, tag="sf")
            for half in range(2):
                sl = slice(2 * SC * half, 2 * SC * (half + 1))
                load_bf16(cf[:, sl], cf_d[:, sl])
                load_bf16(sf[:, sl], sf_d[:, sl])
            wo_t = []
            for k in (range(HD // P) if phase == "full" else []):
                t = wop.tile([P, D], BF16, tag=f"wot{k}", name=f"wo{k}")
                load_bf16(t[:], wot_d[P * k:P * (k + 1), :])
                wo_t.append(t)

            QT = [qk.tile([P, S], BF16, tag=f"qt{m}", name=f"qt{m}")
                  for m in range(HD // P)]
            KT = [qk.tile([P, S], BF16, tag=f"kt{m}", name=f"kt{m}")
                  for m in range(HD // P)]
            A = [apool.tile([P, S], BF16, tag=f"a{m}", name=f"a{m}")
                 for m in range(HD // P)]

            # ---- projection units (generator-driven so heads can interleave) ----
            def rope(t):
                # rows: [hA.evens | hA.odds | hB.evens | hB.odds] (32 each)
                # t = t*cf + swap32pairs(t)*sf
                sw = swp.tile([P, S], BF16, tag="sw", name="sw")
                for g in range(4):
                    src = (g ^ 1) * 32
                    nc.sync.dma_start(sw[g * 32:(g + 1) * 32, :],
                                      t[src:src + 32, :])
                nc.vector.tensor_tensor(t[:], t[:], cf[:], MULT)
                nc.gpsimd.tensor_tensor(sw[:], sw[:], sf[:], MULT)
                nc.vector.tensor_tensor(t[:], t[:], sw[:], ADD)

            rope_done = [False] * (HD // P)

            # weight-slice DMA groups, prefetched one group ahead of their MMs
            groups = [(dram, m, dst)
                      for m in range(HD // P)
                      for dram, dst in ((wqt_d, QT[m]), (wkt_d, KT[m]))]

            def emit_group_dmas(g):
                dram, m, _ = groups[g]
                ws = []
                for k in range(NDB):
                    w = wmp.tile([P, P], F32R, tag="wm", name=f"wm{k}")
                    nc.sync.dma_start(
                        w[:], dram[P * k:P * (k + 1), P * m:P * (m + 1)])
                    ws.append(w)
                return ws

            def qk_units():
                """Yield after each (weight, m, n) projection unit."""
                pre = [emit_group_dmas(0)]
                for g, (dram, m, dst) in enumerate(groups):
                    ws = pre.pop(0)
                    for n in range(NSC):
                        if n == 1 and g + 1 < len(groups):
                            pre.append(emit_group_dmas(g + 1))  # prefetch
                        ps = psp.tile([P, 2 * SC], F32, tag="ps", bufs=2)
                        for k in range(NDB):
                            nc.tensor.matmul(
                                ps[:, 0:SC],
                                ws[k][:],
                                xt[k][:, SC * n:SC * (n + 1)],
                                start=(k == 0), stop=(k == NDB - 1),
                            )
                        nc.vector.tensor_copy(
                            dst[:, SC * n:SC * (n + 1)], ps[:, 0:SC])
                        yield
                    rope(dst)
                    if dst is KT[m]:
                        rope_done[m] = True
                    yield

            units = qk_units()

            def drain_units(count=None, until_tile=None):
                n = 0
                while True:
                    if until_tile is not None and rope_done[until_tile]:
                        return
                    if count is not None and n >= count:
                        return
                    if next(units, StopIteration) is StopIteration:
                        return
                    n += 1

            # head-tile 0 projections + ropes upfront
            drain_units(until_tile=0)

            # ---- V projection (lazy: emitted inside head 0's j-loop so the
            # PE has dense work while ACT runs head-0 exps) ----
            wv_t = []
            for k in range(NDB):
                w = wvp.tile([P, HD], F32R, tag="wv", name=f"wv{k}")
                nc.sync.dma_start(w[:], wvt_d[P * k:P * (k + 1), :])
                wv_t.append(w)
            Vg = [None] * NKB

            def ensure_v(j):
                if Vg[j] is not None:
                    return
                vt = vgp.tile([P, NH * (DH + 1)], BF16, tag=f"vg{j}",
                              name=f"vg{j}")
                v3 = vt[:].rearrange("p (h c) -> p h c", h=NH)
                nc.vector.memset(v3[:, :, DH:DH + 1], 1.0)
                ps = psp.tile([P, 2 * SC], F32, tag="ps", bufs=2)
                for k in range(NDB):
                    nc.tensor.matmul(
                        ps[:, 0:HD],
                        xt[k][:, P * j:P * (j + 1)],
                        wv_t[k][:],
                        start=(k == 0), stop=(k == NDB - 1),
                    )
                nc.vector.tensor_copy(
                    v3[:, :, 0:DH],
                    ps[:, 0:HD].rearrange("p (h c) -> p h c", h=NH))
                Vg[j] = vt

            # ---- debug dumps ----
            def dump_bf16(src, row0):
                # src [128, S] bf16 -> out_d rows row0:row0+128 as f32
                for npair in range(2):
                    sl = slice(2 * SC * npair, 2 * SC * (npair + 1))
                    ot = otp.tile([P, 2 * SC], F32, tag="ot", name="dbg")
                    nc.vector.tensor_copy(ot[:], src[:, sl])
                    nc.sync.dma_start(out_d[row0:row0 + P, sl], ot[:])

            if phase == "proj":
                drain_units()
                for m in range(HD // P):
                    dump_bf16(QT[m], P * m)
                    dump_bf16(KT[m], HD + P * m)

            # ---- attention: flat (h, q-half, k-block) stream with a global
            # software pipeline so the PE never head-of-line blocks across
            # half/head boundaries ----
            if phase == "raw":
                dnall = [apool.tile([1, S], BF16, tag=f"dn{i}", name="dnall")
                         for i in range(2)]
            avs = {}         # (h, HF) -> {qc: av psum tile}

            def normalize(h, ht, ho, qc, avqc):
                # A[h, qc] = av[0:64] * broadcast(1/av[64])
                if phase == "raw":
                    nc.vector.tensor_copy(
                        A[ht][ho:ho + DH, SC * qc:SC * (qc + 1)], avqc[0:DH, :])
                    if h < 2:
                        nc.vector.tensor_copy(
                            dnall[h][:, SC * qc:SC * (qc + 1)],
                            avqc[DH:DH + 1, :])
                    return
                dn = small.tile([1, SC], F32, tag="dn", bufs=2, name="dn")
                nc.vector.tensor_copy(dn[:], avqc[DH:DH + 1, :])
                rcp = small.tile([1, SC], F32, tag="rcp", bufs=2, name="rcp")
                nc.vector.reciprocal_approx_fast(rcp[:], dn[:])
                bc = small.tile([DH, SC], F32, tag="bc", bufs=2, name="bc")
                nc.gpsimd.partition_broadcast(bc[:], rcp[:])
                nc.vector.tensor_tensor(
                    A[ht][ho:ho + DH, SC * qc:SC * (qc + 1)],
                    avqc[0:DH, :], bc[:], MULT)

            def emit_scores(it):
                h, ht, ho, HF, j = it
                qlo = 2 * SC * HF
                qcs = (2 * HF, 2 * HF + 1)
                if h == 0:
                    ensure_v(j)
                if not rope_done[ht]:
                    drain_units(until_tile=ht)
                if j == 0:
                    avs[(h, HF)] = {
                        qc: psp.tile([DH + 1, SC], F32, tag="av", bufs=4,
                                     name=f"av{qc}") for qc in qcs}
                q0 = max(P * j, qlo)
                off = q0 - qlo
                ps = psp.tile([P, 2 * SC], F32, tag="ps", bufs=2)
                for qc in qcs:
                    cs, ce = max(q0, SC * qc), SC * (qc + 1)
                    if cs >= ce:
                        continue
                    nc.tensor.matmul(
                        ps[:, cs - qlo:ce - qlo],
                        KT[ht][ho:ho + DH, P * j:P * (j + 1)],
                        QT[ht][ho:ho + DH, cs:ce],
                        start=True, stop=True,
                    )
                at = atp.tile([P, 2 * SC], BF16, tag="at", name="at")
                nc.scalar.activation(at[:, off:2 * SC], ps[:, off:2 * SC],
                                     EXP, scale=float(SCALE))
                if q0 == P * j:          # diagonal block: keep k <= q
                    nc.vector.tensor_tensor(
                        at[:, off:off + P], at[:, off:off + P], tri[:], MULT)
                return at

            def emit_attnv(it, at):
                h, ht, ho, HF, j = it
                qlo = 2 * SC * HF
                q0 = max(P * j, qlo)
                av = avs[(h, HF)]
                for qc in (2 * HF, 2 * HF + 1):
                    cs, ce = max(q0, SC * qc), SC * (qc + 1)
                    if cs >= ce:
                        continue
                    nc.tensor.matmul(
                        av[qc][:, cs - SC * qc:ce - SC * qc],
                        Vg[j][:, (DH + 1) * h:(DH + 1) * (h + 1)],
                        at[:, cs - qlo:ce - qlo],
                        start=(j == 0), stop=(j == 4 * qc + 3),
                    )
                    if j == 4 * qc + 3:
                        normalize(h, ht, ho, qc, av[qc])  # frees the bank

            LAG = 2
            pending = []
            fill_ctr = 0
            for h in (range(NH) if phase != "proj" else []):
                ht, ho = divmod(h, 2)
                ho *= DH
                for HF in range(2):
                    for j in range(8 * HF + 8):
                        it = (h, ht, ho, HF, j)
                        pending.append((it, emit_scores(it)))
                        if len(pending) > LAG:
                            emit_attnv(*pending.pop(0))
                        fill_ctr += 1
                        if fill_ctr % 6 == 0:
                            drain_units(count=1)
            for args in pending:
                emit_attnv(*args)

            drain_units()  # safety: nothing should remain

            if phase in ("attn", "raw"):
                for m in range(HD // P):
                    dump_bf16(A[m], P * m)
                if phase == "raw":
                    for i in range(2):
                        for npair in range(2):
                            sl = slice(2 * SC * npair, 2 * SC * (npair + 1))
                            ot = otp.tile([1, 2 * SC], F32, tag="ot",
                                          padded_shape=[P, 2 * SC], name="dbgd")
                            nc.vector.tensor_copy(ot[:], dnall[i][:, sl])
                            nc.sync.dma_start(out_d[HD + i:HD + i + 1, sl], ot[:])

            # ---- output projection (row-parallel partial) ----
            for m in (range(D // P) if phase == "full" else []):
                for npair in range(2):
                    ps = psp.tile([P, 2 * SC], F32, tag="ps", bufs=2)
                    for half in range(2):
                        n = 2 * npair + half
                        for k in range(HD // P):
                            nc.tensor.matmul(
                                ps[:, SC * half:SC * (half + 1)],
                                wo_t[k][:, P * m:P * (m + 1)],
                                A[k][:, SC * n:SC * (n + 1)],
                                start=(k == 0), stop=(k == HD // P - 1),
                            )
                    ot = otp.tile([P, 2 * SC], F32, tag="ot", name="ot")
                    nc.vector.tensor_copy(ot[:], ps[:])
                    nc.sync.dma_start(
                        out_d[P * m:P * (m + 1), 2 * SC * npair:2 * SC * (npair + 1)],
                        ot[:])

    nc.compile()
    return nc


_NC_CACHE = []


def _get_nc():
    if not _NC_CACHE:
        _NC_CACHE.append(build_program())
    return _NC_CACHE[0]


def _host_tables(token_positions):
    pos = np.asarray(token_positions).astype(np.float32)
    inv_freq = np.float32(THETA) ** (
        -np.arange(0, DH, 2, dtype=np.float32) / np.float32(DH))
    ang = pos[:, None] * inv_freq[None, :]            # [S, 32] f32
    cos_t = np.ascontiguousarray(np.cos(ang).T)        # [32, S]
    sin_t = np.ascontiguousarray(np.sin(ang).T)
    cfull = np.tile(cos_t, (4, 1)).astype(np.float32)  # [128, S]
    sfull = np.concatenate([-sin_t, sin_t, -sin_t, sin_t], 0).astype(np.float32)
    return cfull, sfull


def _make_in_maps(in_features, token_positions, wq, wk, wv, wo):
    x = np.asarray(in_features, dtype=np.float32)
    wq = np.asarray(wq, dtype=np.float32)
    wk = np.asarray(wk, dtype=np.float32)
    wv = np.asarray(wv, dtype=np.float32)
    wo = np.asarray(wo, dtype=np.float32)

    cfull, sfull = _host_tables(token_positions)
    tri = np.triu(np.ones((P, P), dtype=np.float32))

    # per-head row permutation: evens then odds
    perm1 = np.concatenate([np.arange(0, DH, 2), np.arange(1, DH, 2)])
    perm = np.concatenate([h * DH + perm1 for h in range(NH)])

    in_maps = []
    for c in range(NCORES):
        b, hg = divmod(c, HG)
        sl = slice(hg * HD, (hg + 1) * HD)
        in_maps.append({
            "xt": np.ascontiguousarray(x[b].T),
            "wqt": np.ascontiguousarray(wq[sl][perm].T),
            "wkt": np.ascontiguousarray(wk[sl][perm].T),
            "wvt": np.ascontiguousarray(wv[sl].T),
            "wot": np.ascontiguousarray(wo[:, sl].T),
            "cfull": cfull,
            "sfull": sfull,
            "tri": tri,
        })
    return in_maps


def kernel(in_features, token_positions, wq, wk, wv, wo):
    _install_ntff_hook()
    in_maps = _make_in_maps(in_features, token_positions, wq, wk, wv, wo)
    nc = _get_nc()
    res = run_bass_kernel_spmd(nc, in_maps, list(range(NCORES)))

    out = np.empty((B, S, D), dtype=np.float32)
    for b in range(B):
        acc = res.results[2 * b]["outT"] + res.results[2 * b + 1]["outT"]
        out[b] = acc.T
    return out
